# revision 1
# baseline (speedup 1.0000x reference)
"""Transformer-XL relative-position multi-head attention on 8 Trainium2 cores.

Sharding: tensor-parallel over heads (2 heads/core) for projections+attention,
then AllToAll to redistribute awv^T head-major -> token-sharded, out-projection
+ residual + LayerNorm token-sharded (512 tokens/core).

The rel-shift is computed by bouncing the relative-coordinate position scores
R = (q+v) @ p^T (bf16) through DRAM and reading them back with a skewed +
transposed DMA:  shifted[i, j] = R_flat[i*2047 + 1023 + j]   (j - i <= 1024)
                 0                                           (j - i == 1025)
                 R_flat[i*2047 + 1022 + j]                   (j - i >= 1026)
Scores live in (key, query) layout so softmax needs no transposes: the exp is
unnormalized, a ones-column in the AV matmul produces the softmax denominator,
and the division is applied to awv^T (65 x 512 per head) after the fact.
"""
import numpy as np
import ml_dtypes

import concourse.bass as bass
import concourse.mybir as mybir
import concourse.tile as tile
from concourse import bacc
from concourse.bass_utils import run_bass_kernel_spmd
from concourse.masks import make_identity
import bass_rust

BF = mybir.dt.bfloat16
F32 = mybir.dt.float32
AF = mybir.ActivationFunctionType
ALU = mybir.AluOpType
bf16 = ml_dtypes.bfloat16

S = 1024
PREV = 1024
T = 2048
B = 4
D = 1024
H = 16
d = 64
NC = 8
ST = S * T
SCALE = 1.0 / 8.0
LN_EPS = 1e-5


def _ap(handle, offset, pattern):
    return bass_rust.AP(tensor=handle, offset=offset, ap=pattern)


VARIANT = 0


def _body(nc, tc, io):
    out_t = io["out"]
    r_t = io["r_scratch"]
    a2a_in_t = io["a2a_in"]
    a2a_out_t = io["a2a_out"]

    with tc.tile_pool(name="res", bufs=1) as res:
        # ---- persistent tiles ----
        kt = res.tile([128, B * T], BF, tag="kt")          # k^T, (2*d, b-major tokens)
        vsb = res.tile([128, 64 * 130], BF, tag="vsb")     # [1|v_h0|1|v_h1] per j-tile
        qu = res.tile([128, B * S], BF, tag="qu")
        qv = res.tile([128, B * S], BF, tag="qv")
        pt = res.tile([128, T], BF, tag="pt")
        wout = res.tile([128, 8 * D], BF, tag="wout")
        wk = res.tile([128, D], BF, tag="wk")
        wv = res.tile([128, D], BF, tag="wv")
        wq = res.tile([128, D], BF, tag="wq")
        wp = res.tile([128, D], BF, tag="wp")
        awvt0 = res.tile([64, B * S], BF, tag="awvt0")
        awvt1 = res.tile([64, B * S], BF, tag="awvt1")
        awvf = res.tile([128, 8 * 512], BF, tag="awvf")    # gathered awv^T K-tiles
        ident = res.tile([128, 128], BF, tag="ident")
        ub = res.tile([128, 1], F32, tag="ub")
        vb = res.tile([128, 1], F32, tag="vb")
        lng_r = res.tile([1, D], F32, tag="lngr")
        lnb_r = res.tile([1, D], F32, tag="lnbr")
        lng_b = res.tile([128, D], F32, tag="lngb")
        lnb_b = res.tile([128, D], F32, tag="lnbb")

        # ---- constant loads ----
        for wt_sb, wt_h in ((wk, io["wk"]), (wv, io["wv"]), (wq, io["wq"]), (wp, io["wp"])):
            nc.sync.dma_start(wt_sb[:], _ap(wt_h, 0, [[128, 128], [16384, 8], [1, 128]]))
        nc.sync.dma_start(wout[:], _ap(io["wout"], 0, [[1024, 128], [131072, 8], [1, 1024]]))
        nc.sync.dma_start(ub[:], io["ub"][:])
        nc.sync.dma_start(vb[:], io["vb"][:])
        nc.sync.dma_start(lng_r[:], io["lng"][:])
        nc.sync.dma_start(lnb_r[:], io["lnb"][:])
        nc.gpsimd.partition_broadcast(lng_b[:], lng_r[:])
        nc.gpsimd.partition_broadcast(lnb_b[:], lnb_r[:])
        make_identity(nc, ident[:])
        nc.gpsimd.memset(vsb[:], 1.0)

        # ---- phase 1: projections ----
        with tc.tile_pool(name="xt", bufs=3) as xtp, \
             tc.tile_pool(name="ps1", bufs=3, space="PSUM") as ps1, \
             tc.tile_pool(name="psv", bufs=2, space="PSUM") as psv:
            for nt in range(16):          # 512-token slices, b-major (b = nt // 4)
                xtile = xtp.tile([128, 8 * 512], BF, tag="xt")
                nc.gpsimd.dma_start(
                    xtile[:],
                    _ap(io["xt"], nt * 512,
                        [[B * T, 128], [B * T * 128, 8], [1, 512]]))
                # k^T
                ps = ps1.tile([128, 512], F32, tag="mm")
                for kd in range(8):
                    nc.tensor.matmul(
                        ps[:], lhsT=wk[:, kd * 128:(kd + 1) * 128],
                        rhs=xtile[:, kd * 512:(kd + 1) * 512],
                        start=(kd == 0), stop=(kd == 7))
                nc.scalar.activation(kt[:, nt * 512:(nt + 1) * 512], ps[:], AF.Copy)
                # v (token-partition layout), 4 sub-tiles of 128 tokens
                for sub in range(4):
                    pv = psv.tile([128, 128], F32, tag="v")
                    for kd in range(8):
                        nc.tensor.matmul(
                            pv[:],
                            lhsT=xtile[:, kd * 512 + sub * 128: kd * 512 + (sub + 1) * 128],
                            rhs=wv[:, kd * 128:(kd + 1) * 128],
                            start=(kd == 0), stop=(kd == 7))
                    g = nt * 4 + sub
                    nc.scalar.activation(
                        vsb[:, g * 130: g * 130 + 64], pv[:, 0:64], AF.Copy)
                    nc.scalar.activation(
                        vsb[:, g * 130 + 65: g * 130 + 129], pv[:, 64:128], AF.Copy)
                # q (input_ tokens only: last 1024 of each batch's 2048)
                if nt % 4 >= 2:
                    pq = ps1.tile([128, 512], F32, tag="mm")
                    for kd in range(8):
                        nc.tensor.matmul(
                            pq[:], lhsT=wq[:, kd * 128:(kd + 1) * 128],
                            rhs=xtile[:, kd * 512:(kd + 1) * 512],
                            start=(kd == 0), stop=(kd == 7))
                    qc = (nt // 4) * 1024 + (nt % 4 - 2) * 512
                    nc.scalar.activation(qu[:, qc:qc + 512], pq[:], AF.Identity,
                                         bias=ub[:])
                    nc.scalar.activation(qv[:, qc:qc + 512], pq[:], AF.Identity,
                                         bias=vb[:])
            # p^T
            for rt in range(4):
                ptile = xtp.tile([128, 8 * 512], BF, tag="xt")
                nc.sync.dma_start(
                    ptile[:],
                    _ap(io["pt"], rt * 512,
                        [[T, 128], [T * 128, 8], [1, 512]]))
                pp = ps1.tile([128, 512], F32, tag="mm")
                for kd in range(8):
                    nc.tensor.matmul(
                        pp[:], lhsT=wp[:, kd * 128:(kd + 1) * 128],
                        rhs=ptile[:, kd * 512:(kd + 1) * 512],
                        start=(kd == 0), stop=(kd == 7))
                nc.scalar.activation(pt[:, rt * 512:(rt + 1) * 512], pp[:], AF.Copy)

        # ---- phase 2+3: R bounce, then attention ----
        with tc.tile_pool(name="rst", bufs=4) as rst, \
             tc.tile_pool(name="skew", bufs=8) as skp, \
             tc.tile_pool(name="msk", bufs=2) as mkp, \
             tc.tile_pool(name="attn", bufs=20) as atp, \
             tc.tile_pool(name="nrm", bufs=2) as nrm, \
             tc.tile_pool(name="ps2", bufs=3, space="PSUM") as ps2, \
             tc.tile_pool(name="psr", bufs=2, space="PSUM") as psr, \
             tc.tile_pool(name="psav", bufs=2, space="PSUM") as psav, \
             tc.tile_pool(name="pssum", bufs=1, space="PSUM") as pssum:

            # R = (q+v) @ p^T in (i, r) coords -> DRAM (bf16, flat per bh)
            for b in range(B):
                for hl in range(2):
                    bh = b * 2 + hl
                    hs = slice(hl * 64, (hl + 1) * 64)
                    for it in range(8):
                        rs = rst.tile([128, T], BF, tag="rs")
                        for rt in range(4):
                            pr = psr.tile([128, 512], F32, tag="rsc")
                            nc.tensor.matmul(
                                pr[:],
                                lhsT=qv[hs, b * S + it * 128: b * S + (it + 1) * 128],
                                rhs=pt[hs, rt * 512:(rt + 1) * 512],
                                start=True, stop=True)
                            if it % 2 == 0:
                                nc.scalar.activation(
                                    rs[:, rt * 512:(rt + 1) * 512], pr[:], AF.Copy)
                            else:
                                nc.vector.tensor_copy(
                                    rs[:, rt * 512:(rt + 1) * 512], pr[:])
                        nc.gpsimd.dma_start(
                            _ap(r_t, bh * ST + it * 128 * T, [[T, 128], [1, T]]),
                            rs[:])

            # attention
            for b in range(B):
                for hl in range(2):
                    bh = b * 2 + hl
                    hs = slice(hl * 64, (hl + 1) * 64)
                    for i0b in range(2):
                        i0 = i0b * 512
                        atts = []
                        for t in range(16):
                            ps = ps2.tile([128, 512], F32, tag="sc")
                            nc.tensor.matmul(
                                ps[:],
                                lhsT=kt[hs, b * T + t * 128: b * T + (t + 1) * 128],
                                rhs=qu[hs, b * S + i0: b * S + i0 + 512],
                                start=True, stop=False)
                            dmin = 128 * t - i0 - 511
                            dmax = 128 * t + 127 - i0
                            need1 = dmin <= 1024
                            need2 = dmax >= 1026
                            d1 = d2 = None
                            if need1:
                                d1 = skp.tile([128, 512], BF, tag="d1")
                                if VARIANT == 1:
                                    nc.sync.dma_start(
                                        d1[:],
                                        _ap(r_t, bh * ST + i0 * 512 + 128 * t,
                                            [[512, 128], [1, 512]]))
                                else:
                                    nc.sync.dma_start_transpose(
                                        d1[:],
                                        _ap(r_t, bh * ST + i0 * 2047 + 1023 + 128 * t,
                                            [[2047, 512], [1, 128]]))
                            if need2:
                                d2 = skp.tile([128, 512], BF, tag="d2")
                                if VARIANT == 1:
                                    nc.sync.dma_start(
                                        d2[:],
                                        _ap(r_t, bh * ST + i0 * 512 + 128 * t,
                                            [[512, 128], [1, 512]]))
                                else:
                                    nc.scalar.dma_start_transpose(
                                        d2[:],
                                        _ap(r_t, bh * ST + i0 * 2047 + 1022 + 128 * t,
                                            [[2047, 512], [1, 128]]))
                            if dmax <= 1024:          # pure D1
                                nc.tensor.matmul(ps[:], lhsT=ident[:], rhs=d1[:],
                                                 start=False, stop=True)
                            elif dmin >= 1026:        # pure D2
                                nc.tensor.matmul(ps[:], lhsT=ident[:], rhs=d2[:],
                                                 start=False, stop=True)
                            else:                     # mixed: mask then add
                                pieces = []
                                if need1:
                                    m1 = mkp.tile([128, 512], BF, tag="m1")
                                    nc.gpsimd.affine_select(
                                        out=m1[:], in_=d1[:],
                                        pattern=[[1, 512]],
                                        base=i0 + 1024 - 128 * t,
                                        channel_multiplier=-1,
                                        compare_op=ALU.is_ge, fill=0.0)
                                    pieces.append(m1)
                                if need2:
                                    m2 = mkp.tile([128, 512], BF, tag="m2")
                                    nc.gpsimd.affine_select(
                                        out=m2[:], in_=d2[:],
                                        pattern=[[-1, 512]],
                                        base=128 * t - i0 - 1026,
                                        channel_multiplier=1,
                                        compare_op=ALU.is_ge, fill=0.0)
                                    pieces.append(m2)
                                if len(pieces) == 2:
                                    ms = mkp.tile([128, 512], BF, tag="ms")
                                    nc.vector.tensor_add(ms[:], pieces[0][:],
                                                         pieces[1][:])
                                    nc.tensor.matmul(ps[:], lhsT=ident[:], rhs=ms[:],
                                                     start=False, stop=True)
                                else:
                                    nc.tensor.matmul(ps[:], lhsT=ident[:],
                                                     rhs=pieces[0][:],
                                                     start=False, stop=True)
                            at = atp.tile([128, 512], BF, tag="at")
                            nc.scalar.activation(at[:], ps[:], AF.Exp, scale=SCALE)
                            atts.append(at)
                        # AV with leading ones column -> row 0 = sum(exp)
                        pav = psav.tile([65, 512], F32, tag="av")
                        for t in range(16):
                            g = b * 16 + t
                            nc.tensor.matmul(
                                pav[:],
                                lhsT=vsb[:, g * 130 + hl * 65: g * 130 + hl * 65 + 65],
                                rhs=atts[t][:],
                                start=(t == 0), stop=(t == 15))
                        awvu = nrm.tile([65, 512], BF, tag="awvu")
                        nc.scalar.activation(awvu[:], pav[:], AF.Copy)
                        psum = pssum.tile([1, 512], F32, tag="sum")
                        nc.tensor.matmul(psum[:], lhsT=ident[0:65, 64:65],
                                         rhs=awvu[:], start=True, stop=True)
                        rec = nrm.tile([1, 512], F32, tag="rec")
                        nc.vector.reciprocal(rec[:], psum[:])
                        recb = nrm.tile([64, 512], F32, tag="recb")
                        nc.gpsimd.partition_broadcast(recb[:], rec[:])
                        awvt = awvt0 if hl == 0 else awvt1
                        nc.vector.tensor_tensor(
                            out=awvt[:, b * S + i0: b * S + i0 + 512],
                            in0=awvu[0:64, :], in1=recb[:], op=ALU.mult)

        # ---- phase 4: A2A, out-projection, residual, LayerNorm ----
        nc.sync.dma_start(
            _ap(a2a_in_t, 0, [[512, 64], [65536, 8], [1, 512]]),
            awvt0[:])
        nc.sync.dma_start(
            _ap(a2a_in_t, 64 * 512, [[512, 64], [65536, 8], [1, 512]]),
            awvt1[:])
        if io.get("no_cc"):
            nc.sync.dma_start(a2a_out_t[:], a2a_in_t[:])
        else:
            nc.gpsimd.collective_compute(
                "AllToAll", ALU.bypass,
                replica_groups=[list(range(NC))],
                ins=[a2a_in_t[:]], outs=[a2a_out_t[:]],
            )
        nc.sync.dma_start(awvf[:], _ap(a2a_out_t, 0, [[512, 128], [65536, 8], [1, 512]]))

        with tc.tile_pool(name="outp", bufs=2) as op_, \
             tc.tile_pool(name="stat", bufs=2) as stp, \
             tc.tile_pool(name="ps3", bufs=2, space="PSUM") as ps3:
            for tt in range(4):
                resid = op_.tile([128, D], F32, tag="resid")
                nc.sync.dma_start(resid[:], io["resid"][tt * 128:(tt + 1) * 128, :])
                o = op_.tile([128, D], F32, tag="o")
                for n2 in range(2):
                    po = ps3.tile([128, 512], F32, tag="mm")
                    for kd in range(8):
                        nc.tensor.matmul(
                            po[:],
                            lhsT=awvf[:, kd * 512 + tt * 128: kd * 512 + (tt + 1) * 128],
                            rhs=wout[:, kd * D + n2 * 512: kd * D + n2 * 512 + 512],
                            start=(kd == 0), stop=(kd == 7))
                    nc.vector.tensor_add(
                        o[:, n2 * 512:(n2 + 1) * 512], po[:],
                        resid[:, n2 * 512:(n2 + 1) * 512])
                # LayerNorm over D
                sm = stp.tile([128, 1], F32, tag="sm")
                nc.vector.tensor_reduce(sm[:], o[:], axis=mybir.AxisListType.X,
                                        op=ALU.add)
                mean = stp.tile([128, 1], F32, tag="mean")
                nc.vector.tensor_scalar_mul(mean[:], sm[:], 1.0 / D)
                cent = op_.tile([128, D], F32, tag="cent")
                nc.vector.tensor_scalar(out=cent[:], in0=o[:], scalar1=mean[:],
                                        scalar2=None, op0=ALU.subtract)
                sq = op_.tile([128, D], F32, tag="sq")
                ssq = stp.tile([128, 1], F32, tag="ssq")
                nc.scalar.activation(sq[:], cent[:], AF.Square, accum_out=ssq[:])
                veps = stp.tile([128, 1], F32, tag="veps")
                nc.vector.tensor_scalar(out=veps[:], in0=ssq[:], scalar1=1.0 / D,
                                        scalar2=LN_EPS, op0=ALU.mult, op1=ALU.add)
                std = stp.tile([128, 1], F32, tag="std")
                nc.scalar.activation(std[:], veps[:], AF.Sqrt)
                rstd = stp.tile([128, 1], F32, tag="rstd")
                nc.vector.reciprocal(rstd[:], std[:])
                y1 = op_.tile([128, D], F32, tag="o")
                nc.vector.scalar_tensor_tensor(
                    out=y1[:], in0=cent[:], scalar=rstd[:], in1=lng_b[:],
                    op0=ALU.mult, op1=ALU.mult)
                yf = op_.tile([128, D], F32, tag="cent")
                nc.vector.tensor_add(yf[:], y1[:], lnb_b[:])
                nc.sync.dma_start(out_t[tt * 128:(tt + 1) * 128, :], yf[:])


_compiled = None


def _build(no_cc=False):
    nc = bacc.Bacc("TRN2", target_bir_lowering=False, debug=False, num_devices=NC)
    io = {}
    io["xt"] = nc.dram_tensor("xt", [D, B * T], BF, kind="ExternalInput")
    io["pt"] = nc.dram_tensor("pt", [D, T], BF, kind="ExternalInput")
    io["wk"] = nc.dram_tensor("wk", [D, 128], BF, kind="ExternalInput")
    io["wv"] = nc.dram_tensor("wv", [D, 128], BF, kind="ExternalInput")
    io["wq"] = nc.dram_tensor("wq", [D, 128], BF, kind="ExternalInput")
    io["wp"] = nc.dram_tensor("wp", [D, 128], BF, kind="ExternalInput")
    io["wout"] = nc.dram_tensor("wout", [H * d, D], BF, kind="ExternalInput")
    io["ub"] = nc.dram_tensor("ub", [128, 1], F32, kind="ExternalInput").ap()
    io["vb"] = nc.dram_tensor("vb", [128, 1], F32, kind="ExternalInput").ap()
    io["lng"] = nc.dram_tensor("lng", [1, D], F32, kind="ExternalInput").ap()
    io["lnb"] = nc.dram_tensor("lnb", [1, D], F32, kind="ExternalInput").ap()
    io["resid"] = nc.dram_tensor("resid", [512, D], F32, kind="ExternalInput").ap()
    io["out"] = nc.dram_tensor("out", [512, D], F32, kind="ExternalOutput").ap()
    r_h = nc.dram_tensor("r_scratch", [NC, ST], BF)
    a2a_i = nc.dram_tensor("a2a_in", [NC, 128, 512], BF)
    a2a_o = nc.dram_tensor("a2a_out", [NC, 128, 512], BF)
    io["r_scratch"] = r_h
    io["a2a_in"] = a2a_i
    io["a2a_out"] = a2a_o
    io["no_cc"] = no_cc
    with tile.TileContext(nc) as tc:
        _body(nc, tc, io)
    nc.compile()
    return nc


def _shard(inputs):
    x = np.asarray(inputs["input_"], np.float32)
    pos = np.asarray(inputs["pos_embs"], np.float32)
    mem = np.asarray(inputs["memory"], np.float32)
    u = np.asarray(inputs["u"], np.float32).reshape(-1)
    v = np.asarray(inputs["v"], np.float32).reshape(-1)
    W_kv = np.asarray(inputs["W_kv"], np.float32)
    W_q = np.asarray(inputs["W_q"], np.float32)
    W_p = np.asarray(inputs["W_p"], np.float32)
    W_out = np.asarray(inputs["W_out"], np.float32)
    lng = np.asarray(inputs["ln_g"], np.float32).reshape(1, D)
    lnb = np.asarray(inputs["ln_b"], np.float32).reshape(1, D)

    x_mem = np.concatenate([mem, x], axis=0)                  # (T, B, D)
    xt = np.ascontiguousarray(
        x_mem.transpose(2, 1, 0).reshape(D, B * T)            # (D, b-major tokens)
    ).astype(bf16)
    pt = np.ascontiguousarray(pos.T).astype(bf16)             # (D, T)
    wout_b = W_out.astype(bf16)

    in_maps = []
    for c in range(NC):
        hs = slice(c * 128, (c + 1) * 128)
        b, i0 = c // 2, (c % 2) * 512
        in_maps.append({
            "xt": xt,
            "pt": pt,
            "wk": np.ascontiguousarray(W_kv[:, hs]).astype(bf16),
            "wv": np.ascontiguousarray(W_kv[:, H * d + c * 128: H * d + (c + 1) * 128]).astype(bf16),
            "wq": np.ascontiguousarray(W_q[:, hs]).astype(bf16),
            "wp": np.ascontiguousarray(W_p[:, hs]).astype(bf16),
            "wout": wout_b,
            "ub": np.ascontiguousarray(u[hs].reshape(128, 1)),
            "vb": np.ascontiguousarray(v[hs].reshape(128, 1)),
            "lng": lng,
            "lnb": lnb,
            "resid": np.ascontiguousarray(x[i0:i0 + 512, b, :]),
        })
    return in_maps


LAST_RESULTS = None


def kernel(**inputs):
    global _compiled, LAST_RESULTS
    if _compiled is None:
        _compiled = _build()
    nc = _compiled
    in_maps = _shard(inputs)
    res = run_bass_kernel_spmd(nc, in_maps, core_ids=list(range(NC)))
    LAST_RESULTS = res
    out = np.empty((S, B, D), np.float32)
    for c in range(NC):
        b, i0 = c // 2, (c % 2) * 512
        out[i0:i0 + 512, b, :] = res.results[c]["out"]
    return out

